# revision 2
# baseline (speedup 1.0000x reference)
"""Trainium2 Bass kernel for nn_AttentionEncoderModel (8 NeuronCores).

Strategy: pure data-parallel over batch (B=8 -> 1 element/core), all params
replicated, bf16 TensorE matmuls with fp32 accumulation. Activations are kept
in "transposed" layout [features(partitions), rows(free)] so every matmul is
lhsT = weight tile [K=128, M<=128], rhs = activation tile [K=128, N=512].
The only cross-core communication is an 8-byte AllReduce for the final global
standardization.
"""

import numpy as np
import ml_dtypes

import concourse.bass as bass
import concourse.mybir as mybir
from concourse import bacc
from concourse.tile import TileContext
from concourse.bass_utils import run_bass_kernel_spmd

AF = mybir.ActivationFunctionType
OP = mybir.AluOpType
BF = mybir.dt.bfloat16
F32 = mybir.dt.float32

P = 128
ROWS = 512
B, S, D = 8, 512, 256
H, DH = 8, 32
NB = 8
COMP = 128
LN_EPS = 1e-5
SCALE = 1.0 / np.sqrt(DH)
NEG = -1e9

# fc layer dims (K_in, M_out) and activation
FC_DIMS = [(4096, 4096), (4096, 2048), (2048, 1024), (1024, 512), (512, 256)]


def build_nc(n_cores=8):
    nc = bacc.Bacc("TRN2", target_bir_lowering=False, debug=False,
                   num_devices=n_cores)
    NTOT = float(n_cores * ROWS * COMP)

    # ---------------- DRAM parameters ----------------
    xT = nc.declare_dram_parameter("xT", [32, P, ROWS], BF, False)
    fc_w, fc_b = [], []
    for i, (kin, mout) in enumerate(FC_DIMS + [(256, 256)]):  # + pre layer
        mt, kt = mout // P, kin // P
        fc_w.append(nc.declare_dram_parameter(f"w{i}", [mt, P, kt * P], BF, False))
        fc_b.append(nc.declare_dram_parameter(f"b{i}", [P, mt], F32, False))
    posT_d = nc.declare_dram_parameter("posT", [2, P, ROWS], F32, False)
    encqk_d = nc.declare_dram_parameter("encqk", [NB, P, 1024], BF, False)
    encv_d = nc.declare_dram_parameter("encv", [NB, P, 512], BF, False)
    encbqk_d = nc.declare_dram_parameter("encbqk", [NB, P, 4], F32, False)
    bvb_d = nc.declare_dram_parameter("bvb", [NB, P, 256], F32, False)
    lngb_d = nc.declare_dram_parameter("lngb", [NB, 2, 512], BF, False)
    rw1_d = nc.declare_dram_parameter("rw1", [NB, P, 2048], BF, False)
    rb1_d = nc.declare_dram_parameter("rb1", [NB, P, 8], F32, False)
    rw2_d = nc.declare_dram_parameter("rw2", [NB, P, 2048], BF, False)
    rb2_d = nc.declare_dram_parameter("rb2", [NB, P, 2], F32, False)
    outw_d = nc.declare_dram_parameter("outw", [P, 256], BF, False)
    outb_d = nc.declare_dram_parameter("outb", [P, 1], F32, False)
    tri_d = nc.declare_dram_parameter("tri", [P, P], F32, False)
    m256_d = nc.declare_dram_parameter("m256", [P, 1], BF, False)
    onesP_d = nc.declare_dram_parameter("onesP", [P, 1], BF, False)
    lnones_d = nc.declare_dram_parameter("lnones", [1, ROWS], BF, False)

    out_d = nc.declare_dram_parameter("out", [P, ROWS], F32, True)

    with TileContext(nc) as tc:
        with (
            tc.tile_pool(name="const", bufs=1) as cpool,
            tc.tile_pool(name="stream", bufs=1) as spool,
            tc.tile_pool(name="wpool", bufs=3) as wpool,
            tc.tile_pool(name="dram", bufs=1, space="DRAM") as dpool,
        ):
            # constants
            tri_sb = cpool.tile([P, P], F32, name="tri_sb")
            nc.sync.dma_start(tri_sb[:], tri_d[:])
            m256_sb = cpool.tile([P, 1], BF, name="m256_sb")
            nc.sync.dma_start(m256_sb[:], m256_d[:])
            onesP_sb = cpool.tile([P, 1], BF, name="onesP_sb")
            nc.sync.dma_start(onesP_sb[:], onesP_d[:])
            lnones_sb = cpool.tile([1, ROWS], BF, name="lnones_sb")
            nc.sync.dma_start(lnones_sb[:], lnones_d[:])

            cconst = cpool.tile([P, 2], F32, name="cconst")
            nc.vector.memset(cconst[:, 0:1], 0.0)
            nc.vector.memset(cconst[:, 1:2], LN_EPS)
            nc.const_aps.aps[(F32, 0.0)] = cconst[:, 0:1]
            nc.const_aps.aps[(F32, LN_EPS)] = cconst[:, 1:2]

            # residual stream x^T [256, 512] f32 as 2 tiles
            xs = []
            for m in range(2):
                t = spool.tile([P, ROWS], F32, name=f"xs_{m}")
                xs.append(t)

            # ---------------- MLP front ----------------
            with tc.tile_pool(name="acts", bufs=1) as apool, \
                 tc.tile_pool(name="mlp_ps", bufs=3, space="PSUM") as mpp:
                cur = []
                for k in range(32):
                    t = apool.tile([P, ROWS], BF, name=f"x0_{k}")
                    nc.sync.dma_start(t[:], xT[k])
                    cur.append(t)

                for i, (kin, mout) in enumerate(FC_DIMS):
                    mt, kt = mout // P, kin // P
                    bias_sb = apool.tile([P, mt], F32, name=f"bias{i}")
                    nc.sync.dma_start(bias_sb[:], fc_b[i][:])
                    act = AF.Tanh if i == 4 else AF.Relu
                    nxt = []
                    for m in range(mt):
                        w_sb = wpool.tile([P, kt * P], BF, tag="wmlp",
                                          name=f"w{i}_{m}")
                        nc.sync.dma_start(w_sb[:], fc_w[i][m])
                        ps = mpp.tile([P, ROWS], F32, tag="mlp", name=f"ps{i}_{m}")
                        for k in range(kt):
                            nc.tensor.matmul(ps[:], w_sb[:, k * P:(k + 1) * P],
                                             cur[k][:], start=(k == 0),
                                             stop=(k == kt - 1))
                        o = apool.tile([P, ROWS], BF, name=f"a{i}_{m}")
                        nc.scalar.activation(o[:], ps[:], act,
                                             bias=bias_sb[:, m:m + 1])
                        nxt.append(o)
                    cur = nxt

                # pre layer -> f32 stream + positional
                posT_sb = apool.tile([P, 2 * ROWS], F32, name="posT_sb")
                posT_v = posT_sb.rearrange("p (m r) -> p m r", m=2)
                nc.sync.dma_start(posT_v[:], posT_d.rearrange("m p r -> p m r"))
                bias_sb = apool.tile([P, 2], F32, name="bias5")
                nc.sync.dma_start(bias_sb[:], fc_b[5][:])
                for m in range(2):
                    w_sb = wpool.tile([P, 2 * P], BF, tag="wmlp", name=f"w5_{m}")
                    nc.sync.dma_start(w_sb[:], fc_w[5][m])
                    ps = mpp.tile([P, ROWS], F32, tag="mlp", name=f"ps5_{m}")
                    for k in range(2):
                        nc.tensor.matmul(ps[:], w_sb[:, k * P:(k + 1) * P],
                                         cur[k][:], start=(k == 0), stop=(k == 1))
                    nc.vector.scalar_tensor_tensor(
                        xs[m][:], ps[:], bias_sb[:, m:m + 1], posT_v[:, m, :],
                        op0=OP.add, op1=OP.add)

            # ---------------- transformer blocks ----------------
            def layernorm(l, which, bpool, xn_out_bf, replace_stream):
                """LN over features (partition dim) of xs; writes bf16 tiles
                xn_out_bf[m]; if replace_stream, also overwrites xs[m] (f32).
                gb_sb row layout: [1, 512] = [gamma(256) | beta(256)]."""
                gb_sb = bpool.tile([1, 512], BF, tag="lngb", name=f"gb_{l}_{which}")
                nc.sync.dma_start(gb_sb[:], lngb_d[l, which])
                with tc.tile_pool(name=f"lnps_{l}_{which}", bufs=1,
                                  space="PSUM") as lpp:
                    mu_ps = lpp.tile([1, ROWS], F32, name=f"mu_{l}_{which}")
                    sq_ps = lpp.tile([1, ROWS], F32, name=f"sq_{l}_{which}")
                    for m in range(2):
                        xbf = bpool.tile([P, ROWS], BF, tag="ln_xbf",
                                         name=f"lnxbf_{l}_{which}_{m}")
                        nc.vector.tensor_copy(xbf[:], xs[m][:])
                        sqbf = bpool.tile([P, ROWS], BF, tag="ln_sqbf",
                                          name=f"lnsq_{l}_{which}_{m}")
                        nc.scalar.activation(sqbf[:], xs[m][:], AF.Square)
                        nc.tensor.matmul(mu_ps[:], m256_sb[:], xbf[:],
                                         start=(m == 0), stop=(m == 1))
                        nc.tensor.matmul(sq_ps[:], m256_sb[:], sqbf[:],
                                         start=(m == 0), stop=(m == 1))
                    # row math
                    t1 = bpool.tile([1, ROWS], F32, tag="ln_t1",
                                    name=f"lnt1_{l}_{which}")
                    nc.scalar.activation(t1[:], mu_ps[:], AF.Square)
                    var = bpool.tile([1, ROWS], F32, tag="ln_var",
                                     name=f"lnvar_{l}_{which}")
                    nc.vector.tensor_tensor(var[:], sq_ps[:], t1[:],
                                            op=OP.subtract)
                    lnv = bpool.tile([1, ROWS], F32, tag="ln_lnv",
                                     name=f"lnlnv_{l}_{which}")
                    nc.scalar.activation(lnv[:], var[:], AF.Ln, bias=LN_EPS)
                    rstd = bpool.tile([1, ROWS], F32, tag="ln_rstd",
                                      name=f"lnrstd_{l}_{which}")
                    nc.scalar.activation(rstd[:], lnv[:], AF.Exp, scale=-0.5)
                    rstd_bf = bpool.tile([1, ROWS], BF, tag="ln_rstdbf",
                                         name=f"lnrstdbf_{l}_{which}")
                    nc.vector.tensor_copy(rstd_bf[:], rstd[:])
                    nmr_bf = bpool.tile([1, ROWS], BF, tag="ln_nmr",
                                        name=f"lnnmr_{l}_{which}")
                    nc.vector.scalar_tensor_tensor(
                        nmr_bf[:], mu_ps[:], -1.0, rstd[:],
                        op0=OP.mult, op1=OP.mult)
                    for m in range(2):
                        a_ps = lpp.tile([P, ROWS], F32, tag="ln_ab", bufs=2,
                                        name=f"lnA_{l}_{which}_{m}")
                        nc.tensor.matmul(a_ps[:], gb_sb[0:1, m * P:(m + 1) * P],
                                         rstd_bf[:], start=True, stop=True)
                        b_ps = lpp.tile([P, ROWS], F32, tag="ln_ab", bufs=2,
                                        name=f"lnB_{l}_{which}_{m}")
                        nc.tensor.matmul(b_ps[:], gb_sb[0:1, m * P:(m + 1) * P],
                                         nmr_bf[:], start=True, stop=False)
                        nc.tensor.matmul(b_ps[:],
                                         gb_sb[0:1, 256 + m * P:256 + (m + 1) * P],
                                         lnones_sb[:], start=False, stop=True)
                        tmp = bpool.tile([P, ROWS], F32, tag="ln_tmp",
                                         name=f"lntmp_{l}_{which}_{m}")
                        nc.vector.tensor_tensor(tmp[:], xs[m][:], a_ps[:],
                                                op=OP.mult)
                        if replace_stream:
                            nc.vector.tensor_tensor(xs[m][:], tmp[:], b_ps[:],
                                                    op=OP.add)
                            nc.vector.tensor_copy(xn_out_bf[m][:], xs[m][:])
                        else:
                            nc.vector.tensor_tensor(xn_out_bf[m][:], tmp[:],
                                                    b_ps[:], op=OP.add)

            for l in range(NB):
                with tc.tile_pool(name=f"blk_{l}", bufs=1) as bpool:
                    # ---- ln1 -> xn1 (bf16 only)
                    xn1 = [bpool.tile([P, ROWS], BF, tag=f"xn1_{m}",
                                      name=f"xn1_{l}_{m}") for m in range(2)]
                    layernorm(l, 0, bpool, xn1, replace_stream=False)

                    # ---- qkv weights for this block
                    eqk_sb = bpool.tile([P, 1024], BF, tag="eqk",
                                        name=f"eqk_{l}")
                    nc.sync.dma_start(eqk_sb[:], encqk_d[l])
                    ev_sb = bpool.tile([P, 512], BF, tag="ev", name=f"ev_{l}")
                    nc.sync.dma_start(ev_sb[:], encv_d[l])
                    ebqk_sb = bpool.tile([P, 4], F32, tag="ebqk",
                                         name=f"ebqk_{l}")
                    nc.sync.dma_start(ebqk_sb[:], encbqk_d[l])
                    bvb_sb = bpool.tile([P, 256], F32, tag="bvb",
                                        name=f"bvb_{l}")
                    nc.sync.dma_start(bvb_sb[:], bvb_d[l])

                    eqk_v = eqk_sb.rearrange("p (m k c) -> p m k c", m=4, k=2)
                    qk_bf = []
                    with tc.tile_pool(name=f"qkps_{l}", bufs=2,
                                      space="PSUM") as qpp:
                        for mt in range(4):
                            ps = qpp.tile([P, ROWS], F32, tag="qk",
                                          name=f"qkps_{l}_{mt}")
                            for k in range(2):
                                nc.tensor.matmul(ps[:], eqk_v[:, mt, k, :],
                                                 xn1[k][:], start=(k == 0),
                                                 stop=(k == 1))
                            o = bpool.tile([P, ROWS], BF, tag=f"qk_{mt}",
                                           name=f"qkbf_{l}_{mt}")
                            nc.scalar.activation(o[:], ps[:], AF.Identity,
                                                 bias=ebqk_sb[:, mt:mt + 1])
                            qk_bf.append(o)
                        # V (natural layout) + aug with ones column
                        ev_v = ev_sb.rearrange("p (k c) -> p k c", k=2)
                        v_aug = []
                        for rt in range(4):
                            ps = qpp.tile([P, 256], F32, tag="v",
                                          name=f"vps_{l}_{rt}")
                            for k in range(2):
                                nc.tensor.matmul(
                                    ps[:], xn1[k][:, rt * P:(rt + 1) * P],
                                    ev_v[:, k, :], start=(k == 0), stop=(k == 1))
                            va = bpool.tile([P, 264], BF, tag=f"vaug_{rt}",
                                            name=f"vaug_{l}_{rt}")
                            va_v = va.rearrange("p (h c) -> p h c", c=33)
                            nc.vector.scalar_tensor_tensor(
                                va_v[:, :, 0:32],
                                ps.rearrange("p (h c) -> p h c", c=32),
                                1.0,
                                bvb_sb.rearrange("p (h c) -> p h c", c=32),
                                op0=OP.mult, op1=OP.add)
                            nc.vector.memset(va_v[:, :, 32:33], 1.0)
                            v_aug.append(va)

                    # ---- attention per head-group (heads 4g..4g+3 -> x tile g)
                    for g in range(2):
                        with tc.tile_pool(name=f"att_{l}_{g}", bufs=1,
                                          space="PSUM") as app:
                            expS = {}
                            for t in range(4):
                                for hh in range(4):
                                    s_ps = app.tile([P, ROWS], F32, tag="s",
                                                    bufs=4,
                                                    name=f"sps_{l}_{g}_{t}_{hh}")
                                    lhsT = qk_bf[2 + g][32 * hh:32 * hh + 32,
                                                        t * P:(t + 1) * P]
                                    rhs = qk_bf[g][32 * hh:32 * hh + 32, :]
                                    nc.tensor.matmul(s_ps[:], lhsT, rhs,
                                                     start=True, stop=True,
                                                     tile_position=(32 * hh, 0))
                                    nc.vector.tensor_tensor(
                                        s_ps[:, t * P:(t + 1) * P],
                                        s_ps[:, t * P:(t + 1) * P],
                                        tri_sb[:], op=OP.add)
                                    e = bpool.tile([P, ROWS], BF,
                                                   tag=f"expS_{hh}_{t}",
                                                   name=f"expS_{l}_{g}_{hh}_{t}")
                                    if t > 0:
                                        nc.vector.memset(e[:, 0:t * P], 0.0)
                                    nc.scalar.activation(
                                        e[:, t * P:], s_ps[:, t * P:], AF.Exp,
                                        scale=SCALE)
                                    expS[(hh, t)] = e
                            dbf = [bpool.tile([1, ROWS], BF, tag=f"dbf_{hh}",
                                              name=f"dbf_{l}_{g}_{hh}")
                                   for hh in range(4)]
                            pv_tiles = []
                            for pi in range(2):
                                hh0, hh1 = 2 * pi, 2 * pi + 1
                                pv = app.tile([P, ROWS], F32, tag="pv", bufs=2,
                                              name=f"pv_{l}_{g}_{pi}")
                                gA, gB = 4 * g + hh0, 4 * g + hh1
                                for t in range(4):
                                    nc.tensor.matmul(
                                        pv[0:33, :],
                                        v_aug[t][:, 33 * gA:33 * gA + 33],
                                        expS[(hh0, t)][:],
                                        start=(t == 0), stop=(t == 3),
                                        tile_position=(0, 0),
                                        skip_group_check=True)
                                    nc.tensor.matmul(
                                        pv[64:97, :],
                                        v_aug[t][:, 33 * gB:33 * gB + 33],
                                        expS[(hh1, t)][:],
                                        start=(t == 0), stop=(t == 3),
                                        tile_position=(0, 64),
                                        skip_group_check=True)
                                for hh, prow in ((hh0, 32), (hh1, 96)):
                                    dr = bpool.tile([1, ROWS], F32, tag="drec",
                                                    name=f"dr_{l}_{g}_{hh}")
                                    nc.vector.reciprocal(dr[:],
                                                         pv[prow:prow + 1, :])
                                    nc.vector.tensor_copy(dbf[hh][:], dr[:])
                                pv_tiles.append(pv)
                            r_ps = app.tile([P, ROWS], F32, tag="r", bufs=1,
                                            name=f"r_{l}_{g}")
                            for q in range(4):
                                nc.tensor.matmul(r_ps[32 * q:32 * q + 32, :],
                                                 lnones_sb[0:1, 0:32],
                                                 dbf[q][:],
                                                 start=True, stop=True,
                                                 tile_position=(0, 32 * q))
                            at_sb = bpool.tile([P, ROWS], F32, tag="at_sb",
                                               name=f"atsb_{l}_{g}")
                            for q in range(4):
                                off = 64 * (q % 2)
                                nc.scalar.activation(
                                    at_sb[32 * q:32 * q + 32, :],
                                    pv_tiles[q // 2][off:off + 32, :], AF.Copy)
                                nc.vector.tensor_tensor(
                                    at_sb[32 * q:32 * q + 32, :],
                                    at_sb[32 * q:32 * q + 32, :],
                                    r_ps[32 * q:32 * q + 32, :], op=OP.mult)
                            nc.vector.tensor_tensor(
                                xs[g][:], xs[g][:], at_sb[:], op=OP.add)

                    # ---- ln2 (replaces stream) -> xn2 bf16
                    xn2 = [bpool.tile([P, ROWS], BF, tag=f"xn2_{m}",
                                      name=f"xn2_{l}_{m}") for m in range(2)]
                    layernorm(l, 1, bpool, xn2, replace_stream=True)

                    # ---- FFN
                    rw1_sb = bpool.tile([P, 2048], BF, tag="rw1",
                                        name=f"rw1_{l}")
                    nc.sync.dma_start(rw1_sb[:], rw1_d[l])
                    rb1_sb = bpool.tile([P, 8], F32, tag="rb1", name=f"rb1_{l}")
                    nc.sync.dma_start(rb1_sb[:], rb1_d[l])
                    rw2_sb = bpool.tile([P, 2048], BF, tag="rw2",
                                        name=f"rw2_{l}")
                    nc.sync.dma_start(rw2_sb[:], rw2_d[l])
                    rb2_sb = bpool.tile([P, 2], F32, tag="rb2", name=f"rb2_{l}")
                    nc.sync.dma_start(rb2_sb[:], rb2_d[l])

                    rw1_v = rw1_sb.rearrange("p (m k c) -> p m k c", m=8, k=2)
                    rw2_v = rw2_sb.rearrange("p (m k c) -> p m k c", m=2, k=8)
                    with tc.tile_pool(name=f"ffps_{l}", bufs=2,
                                      space="PSUM") as fpp:
                        h1 = []
                        for mt in range(8):
                            ps = fpp.tile([P, ROWS], F32, tag="f1",
                                          name=f"f1ps_{l}_{mt}")
                            for k in range(2):
                                nc.tensor.matmul(ps[:], rw1_v[:, mt, k, :],
                                                 xn2[k][:], start=(k == 0),
                                                 stop=(k == 1))
                            o = bpool.tile([P, ROWS], BF, tag=f"h1_{mt}",
                                           name=f"h1_{l}_{mt}")
                            nc.scalar.activation(o[:], ps[:], AF.Gelu,
                                                 bias=rb1_sb[:, mt:mt + 1])
                            h1.append(o)
                        for mt in range(2):
                            ps = fpp.tile([P, ROWS], F32, tag="f2",
                                          name=f"f2ps_{l}_{mt}")
                            for k in range(8):
                                nc.tensor.matmul(ps[:], rw2_v[:, mt, k, :],
                                                 h1[k][:], start=(k == 0),
                                                 stop=(k == 7))
                            nc.vector.scalar_tensor_tensor(
                                xs[mt][:], ps[:], rb2_sb[:, mt:mt + 1],
                                xs[mt][:], op0=OP.add, op1=OP.add)

            # ---------------- output head + global standardize ----------------
            outw_sb = cpool.tile([P, 256], BF, name="outw_sb")
            nc.sync.dma_start(outw_sb[:], outw_d[:])
            outb_sb = cpool.tile([P, 1], F32, name="outb_sb")
            nc.sync.dma_start(outb_sb[:], outb_d[:])
            xfbf = [cpool.tile([P, ROWS], BF, name=f"xfbf_{m}")
                    for m in range(2)]
            for m in range(2):
                nc.vector.tensor_copy(xfbf[m][:], xs[m][:])
            with tc.tile_pool(name="fin_ps", bufs=1, space="PSUM") as opp:
                ops = opp.tile([P, ROWS], F32, name="out_ps")
                for k in range(2):
                    nc.tensor.matmul(ops[:], outw_sb[:, k * P:(k + 1) * P],
                                     xfbf[k][:], start=(k == 0), stop=(k == 1))
                out_sb = cpool.tile([P, ROWS], F32, name="out_sb")
                nc.scalar.activation(out_sb[:], ops[:], AF.Identity,
                                     bias=outb_sb[:, 0:1])
                sc = cpool.tile([P, 2], F32, name="sc")
                nc.vector.tensor_reduce(sc[:, 0:1], out_sb[:],
                                        axis=mybir.AxisListType.X, op=OP.add)
                sq_scr = cpool.tile([P, ROWS], F32, name="sq_scr")
                nc.scalar.activation(sq_scr[:], out_sb[:], AF.Square,
                                     accum_out=sc[:, 1:2])
                scbf = cpool.tile([P, 2], BF, name="scbf")
                nc.vector.tensor_copy(scbf[:], sc[:])
                tot_ps = opp.tile([1, 2], F32, name="tot_ps")
                nc.tensor.matmul(tot_ps[:], onesP_sb[:], scbf[:],
                                 start=True, stop=True)

                tot_sb = cpool.tile([1, 2], F32, name="tot_sb")
                nc.vector.tensor_copy(tot_sb[:], tot_ps[:])
                if n_cores > 1:
                    cc_in = dpool.tile([1, 2], F32, name="cc_in")
                    cc_out = dpool.tile([1, 2], F32, addr_space="Shared",
                                        name="cc_out")
                    nc.sync.dma_start(cc_in[:], tot_sb[:])
                    nc.gpsimd.collective_compute(
                        "AllReduce", OP.add,
                        replica_groups=[list(range(n_cores))],
                        ins=[cc_in[:]], outs=[cc_out[:]])
                    st_sb = cpool.tile([1, 2], F32, name="st_sb")
                    nc.sync.dma_start(st_sb[:], cc_out[:])
                else:
                    st_sb = tot_sb

                mean = cpool.tile([1, 1], F32, name="mean")
                nc.vector.tensor_scalar(mean[:], st_sb[:, 0:1], 1.0 / NTOT,
                                        None, op0=OP.mult)
                tb = cpool.tile([1, 1], F32, name="tb")
                nc.vector.tensor_tensor(tb[:], mean[:], mean[:], op=OP.mult)
                ta = cpool.tile([1, 1], F32, name="ta")
                nc.vector.tensor_scalar(ta[:], st_sb[:, 1:2],
                                        1.0 / (NTOT - 1.0), None, op0=OP.mult)
                var = cpool.tile([1, 1], F32, name="var")
                nc.vector.scalar_tensor_tensor(
                    var[:], tb[:], -NTOT / (NTOT - 1.0), ta[:],
                    op0=OP.mult, op1=OP.add)
                lnv = cpool.tile([1, 1], F32, name="lnv")
                nc.scalar.activation(lnv[:], var[:], AF.Ln)
                rs_pack = cpool.tile([1, 2], F32, name="rs_pack")
                nc.scalar.activation(rs_pack[:, 0:1], lnv[:], AF.Exp,
                                     scale=-0.5)
                tshift = cpool.tile([1, 1], F32, name="tshift")
                nc.vector.scalar_tensor_tensor(
                    tshift[:], mean[:], -1.0, rs_pack[:, 0:1],
                    op0=OP.mult, op1=OP.mult)
                nc.vector.tensor_scalar(rs_pack[:, 1:2], tshift[:], 1e-10,
                                        None, op0=OP.add)
                bc = cpool.tile([P, 2], F32, name="bc")
                nc.gpsimd.partition_broadcast(bc[:], rs_pack[:])
                nc.vector.tensor_scalar(out_sb[:], out_sb[:], bc[:, 0:1],
                                        bc[:, 1:2], op0=OP.mult, op1=OP.add)
                nc.sync.dma_start(out_d[:], out_sb[:])

    nc.compile()
    return nc


# ---------------- host-side weight prep ----------------

def _bf(a):
    return np.ascontiguousarray(a).astype(ml_dtypes.bfloat16)


def _f32(a):
    return np.ascontiguousarray(a, dtype=np.float32)


def _tile_w(w):
    """[K, M] -> [Mt, 128, Kt*128] with sb[m, p, k*128+c] = w[k*128+p, m*128+c]."""
    K, M = w.shape
    kt, mt = K // P, M // P
    return _bf(w.reshape(kt, P, mt, P).transpose(2, 1, 0, 3).reshape(mt, P, kt * P))


def _bias_grid(b):
    """[M] -> [128, Mt] with sb[p, m] = b[m*128+p]."""
    M = b.shape[0]
    return _f32(np.asarray(b).reshape(M // P, P).T)


def prep_shared(inp):
    d = {}
    for i, name in enumerate(["fc1", "fc2", "fc3", "fc4", "fc5"]):
        d[f"w{i}"] = _tile_w(np.asarray(inp[f"{name}_w"]))
        d[f"b{i}"] = _bias_grid(np.asarray(inp[f"{name}_b"]))
    d["w5"] = _tile_w(np.asarray(inp["pre_w"]))
    d["b5"] = _bias_grid(np.asarray(inp["pre_b"]))
    d["posT"] = _f32(np.asarray(inp["pos_w"])[0].T.reshape(2, P, ROWS))

    enc_w = np.asarray(inp["enc_w"])  # [NB, 256, 768]
    enc_b = np.asarray(inp["enc_b"])  # [NB, 768]
    d["encqk"] = _bf(enc_w[:, :, :512].reshape(NB, 2, P, 4, P)
                     .transpose(0, 2, 3, 1, 4).reshape(NB, P, 1024))
    d["encv"] = _bf(enc_w[:, :, 512:].reshape(NB, 2, P, 256)
                    .transpose(0, 2, 1, 3).reshape(NB, P, 512))
    d["encbqk"] = _f32(enc_b[:, :512].reshape(NB, 4, P).transpose(0, 2, 1))
    d["bvb"] = _f32(np.broadcast_to(enc_b[:, None, 512:], (NB, P, 256)))

    lngb = np.stack([
        np.concatenate([np.asarray(inp["ln1_g"]),
                        np.asarray(inp["ln1_b"])], axis=1),
        np.concatenate([np.asarray(inp["ln2_g"]),
                        np.asarray(inp["ln2_b"])], axis=1),
    ], axis=1)  # [NB, 2(which), 512 = g|b]
    d["lngb"] = _bf(lngb)

    rw1 = np.asarray(inp["res_w1"])  # [NB, 256, 1024]
    d["rw1"] = _bf(rw1.reshape(NB, 2, P, 8, P).transpose(0, 2, 3, 1, 4)
                   .reshape(NB, P, 2048))
    d["rb1"] = _f32(np.asarray(inp["res_b1"]).reshape(NB, 8, P)
                    .transpose(0, 2, 1))
    rw2 = np.asarray(inp["res_w2"])  # [NB, 1024, 256]
    d["rw2"] = _bf(rw2.reshape(NB, 8, P, 2, P).transpose(0, 2, 3, 1, 4)
                   .reshape(NB, P, 2048))
    d["rb2"] = _f32(np.asarray(inp["res_b2"]).reshape(NB, 2, P)
                    .transpose(0, 2, 1))
    ow = np.asarray(inp["out_w"])  # [256, 128]
    d["outw"] = _bf(ow.reshape(2, P, P).transpose(1, 0, 2).reshape(P, 256))
    d["outb"] = _f32(np.asarray(inp["out_b"]).reshape(P, 1))

    jj = np.arange(P)[:, None]
    ii = np.arange(P)[None, :]
    d["tri"] = _f32(np.where(ii >= jj, 0.0, NEG))
    d["m256"] = _bf(np.full((P, 1), 1.0 / 256.0))
    d["onesP"] = _bf(np.ones((P, 1)))
    d["lnones"] = _bf(np.ones((1, ROWS)))
    return d


_CACHED_NC = None
TRACE = False
LAST_RESULT = None
LAST_IN_MAPS = None


def kernel(**inputs) -> np.ndarray:
    global _CACHED_NC, LAST_RESULT, LAST_IN_MAPS
    if _CACHED_NC is None:
        _CACHED_NC = build_nc(8)
    nc = _CACHED_NC

    shared = prep_shared(inputs)
    state = np.asarray(inputs["state"], dtype=np.float32).reshape(B, S, 4096)
    in_maps = []
    for b in range(B):
        m = dict(shared)
        m["xT"] = _bf(state[b].T.reshape(32, P, ROWS))
        in_maps.append(m)
    LAST_IN_MAPS = in_maps

    res = run_bass_kernel_spmd(nc, in_maps, core_ids=list(range(8)),
                               trace=TRACE)
    LAST_RESULT = res
    out = np.stack([res.results[i]["out"] for i in range(B)])  # [B, COMP, S]
    return np.ascontiguousarray(out.transpose(0, 2, 1)).astype(np.float32)



# revision 58
# speedup vs baseline: 37.9162x; 37.9162x over previous
"""Trainium2 Bass kernel for nn_AttentionEncoderModel (8 NeuronCores).

Strategy: pure data-parallel over batch (B=8 -> 1 element/core), all params
replicated, bf16 TensorE matmuls with fp32 accumulation. Activations kept
transposed [features(partitions), rows(free)].

Optimizations over the v1 kernel:
- LayerNorm gamma/beta folded into the following matmul weights on the host;
  the per-row mean/rstd correction enters the same PSUM accumulation as a
  rank-2 matmul (lhsT=[colsum(W); folded bias], rhs=[nmr; ones]).
- Causal trim: score/PV matmuls and softmax exp only touch columns >= t*128.
- Causal mask applied as a post-exp 0/1 bf16 multiply (GPSIMD) instead of a
  -1e9 add + memset, removing all attention memsets from DVE.
- Activation-table management: only {Ln, Exp} <-> {Gelu} swaps remain (2 per
  block), prefetched with dummy [1,1] activations so the 1.3us table loads
  hide under matmul phases.
- Work spread across DVE / Act / GPSIMD(Pool) so no single engine stalls PE.
- Block weights double-buffered in a long-lived pool (prefetch 1 block ahead).
- Input state loaded with one large DMA instead of 33 small ones.
- `reps` parameter repeats the whole computation for delta-timing on HW.
"""

import numpy as np
import ml_dtypes

import concourse.bass as bass
import concourse.mybir as mybir
from concourse import bacc
from concourse.tile import TileContext
from concourse.bass_utils import run_bass_kernel_spmd

AF = mybir.ActivationFunctionType
OP = mybir.AluOpType
BF = mybir.dt.bfloat16
F32 = mybir.dt.float32

P = 128
ROWS = 512
B, S, D = 8, 512, 256
H, DH = 8, 32
NB = 8
COMP = 128
LN_EPS = 1e-5
SCALE = 1.0 / np.sqrt(DH)

FC_DIMS = [(4096, 4096), (4096, 2048), (2048, 1024), (1024, 512), (512, 256)]

USE_POOL = True  # use GPSIMD(Pool) engine for bf16 copies/mults


def build_nc(n_cores=8, reps=1):
    nc = bacc.Bacc("TRN2", target_bir_lowering=False, debug=False,
                   num_devices=n_cores)
    NTOT = float(n_cores * ROWS * COMP)

    # ---------------- DRAM parameters ----------------
    xT = nc.declare_dram_parameter("xT", [32, P, ROWS], BF, False)
    fc_w, fc_b = [], []
    for i, (kin, mout) in enumerate(FC_DIMS + [(256, 256)]):  # + pre layer
        mt, kt = mout // P, kin // P
        fc_w.append(nc.declare_dram_parameter(f"w{i}", [mt, P, kt * P], BF, False))
        fc_b.append(nc.declare_dram_parameter(f"b{i}", [P, mt], F32, False))
    posT_d = nc.declare_dram_parameter("posT", [2, P, ROWS], F32, False)
    encqk_d = nc.declare_dram_parameter("encqk", [NB, P, 1024], BF, False)
    qkr2_d = nc.declare_dram_parameter("qkr2", [NB, 33, 512], BF, False)
    encv_d = nc.declare_dram_parameter("encv", [NB, P, 512], BF, False)
    vr2_d = nc.declare_dram_parameter("vr2", [NB, 33, 256], BF, False)
    rw1_d = nc.declare_dram_parameter("rw1", [NB, P, 2048], BF, False)
    f1r2_d = nc.declare_dram_parameter("f1r2", [NB, 33, 1024], BF, False)
    rw2_d = nc.declare_dram_parameter("rw2", [NB, P, 2048], BF, False)
    rb2_d = nc.declare_dram_parameter("rb2", [NB, P, 2], F32, False)
    g2b2_d = nc.declare_dram_parameter("g2b2r", [NB, 33, 256], BF, False)
    g2f_d = nc.declare_dram_parameter("g2f", [NB, 1, 256], F32, False)
    fc1kT_d = nc.declare_dram_parameter("fc1kT", [8, P, 4096], BF, False)
    outw_d = nc.declare_dram_parameter("outw", [P, 256], BF, False)
    outb_d = nc.declare_dram_parameter("outb", [P, 1], F32, False)
    mask_d = nc.declare_dram_parameter("maskbf", [P, P], BF, False)
    m256_d = nc.declare_dram_parameter("m256", [P, 1], BF, False)
    onesP_d = nc.declare_dram_parameter("onesP", [P, 1], BF, False)
    lnones_d = nc.declare_dram_parameter("lnones", [1, ROWS], BF, False)
    rpat_d = nc.declare_dram_parameter("rpat", [33, P], BF, False)

    out_d = nc.declare_dram_parameter("out", [P, ROWS], F32, True)

    with TileContext(nc) as tc:
        with (
            tc.tile_pool(name="const", bufs=1) as cpool,
            tc.tile_pool(name="wpool", bufs=3) as wpool,
            tc.tile_pool(name="dram", bufs=1, space="DRAM") as dpool,
        ):
            # constants
            mask_sb = cpool.tile([P, P], BF, name="mask_sb")
            nc.sync.dma_start(mask_sb[:], mask_d[:])
            m256_sb = cpool.tile([P, 1], BF, name="m256_sb")
            nc.sync.dma_start(m256_sb[:], m256_d[:])
            onesP_sb = cpool.tile([P, 1], BF, name="onesP_sb")
            nc.sync.dma_start(onesP_sb[:], onesP_d[:])
            lnones_sb = cpool.tile([1, ROWS], BF, name="lnones_sb")
            nc.sync.dma_start(lnones_sb[:], lnones_d[:])
            rpat_sb = cpool.tile([33, P], BF, name="rpat_sb")
            nc.sync.dma_start(rpat_sb[:], rpat_d[:])

            cconst = cpool.tile([P, 2], F32, name="cconst")
            nc.vector.memset(cconst[:, 0:1], 0.0)
            nc.vector.memset(cconst[:, 1:2], LN_EPS)
            nc.const_aps.aps[(F32, 0.0)] = cconst[:, 0:1]
            nc.const_aps.aps[(F32, LN_EPS)] = cconst[:, 1:2]
            dummy_sb = cpool.tile([1, 2], F32, name="dummy_sb")
            nc.vector.memset(dummy_sb[:], 0.5)
            onesF = cpool.tile([1, P], F32, name="onesF")
            nc.vector.memset(onesF[:], 1.0)
            nc.onesF = onesF

            for rep in range(reps):
                emit_rep(nc, tc, rep, n_cores, NTOT, wpool, dpool,
                         xT, fc_w, fc_b, posT_d, encqk_d, qkr2_d, encv_d,
                         vr2_d, rw1_d, f1r2_d, rw2_d, rb2_d, g2b2_d, g2f_d,
                         fc1kT_d, outw_d, outb_d, out_d,
                         mask_sb, m256_sb, onesP_sb, lnones_sb, rpat_sb,
                         dummy_sb)

    nc.compile()
    return nc


def emit_rep(nc, tc, rep, n_cores, NTOT, wpool, dpool,
             xT, fc_w, fc_b, posT_d, encqk_d, qkr2_d, encv_d, vr2_d,
             rw1_d, f1r2_d, rw2_d, rb2_d, g2b2_d, g2f_d, fc1kT_d,
             outw_d, outb_d, out_d,
             mask_sb, m256_sb, onesP_sb, lnones_sb, rpat_sb, dummy_sb):
    R = f"r{rep}"
    ones128 = lnones_sb[0:1, 0:P]          # [1,128] ones, bc-matmul lhsT

    def dummy_act(sp, func, name):
        d = sp.tile([1, 1], F32, tag="dummy", name=f"{R}_{name}")
        nc.scalar.activation(d[:], dummy_sb[0:1, 0:1], func)

    with tc.tile_pool(name=f"{R}_stream", bufs=1) as spool:
        # residual stream x^T [256, 512] f32 as 2 tiles + bf16 shadow
        xs = [spool.tile([P, ROWS], F32, name=f"{R}_xs_{m}") for m in range(2)]
        xbf = [spool.tile([P, ROWS], BF, name=f"{R}_xbf_{m}") for m in range(2)]
        # rank-2 rhs tiles [nmr; ones] for LN1 / LN2
        r2b = spool.tile([33, ROWS], BF, name=f"{R}_r2b")
        nc.vector.memset(r2b[:], 0.0)
        nc.vector.memset(r2b[32:33, :], 1.0)
        # [-mu(p0); std(p32)] rank-2 rhs (pre-scale variant, for PSUMs that
        # get a rstd column- or row-scale afterwards); rows 1-31 stay zero
        r2qa = spool.tile([33, ROWS], BF, name=f"{R}_r2qa")
        nc.vector.memset(r2qa[:], 0.0)
        r2qb = spool.tile([33, ROWS], BF, name=f"{R}_r2qb")
        nc.vector.memset(r2qb[:], 0.0)
        dpairs = []
        for i in range(2):
            t = spool.tile([33, ROWS], BF, name=f"{R}_dpair_{i}")
            nc.vector.memset(t[:], 0.0)
            dpairs.append(t)

        # ---------------- MLP front ----------------
        with tc.tile_pool(name=f"{R}_acts", bufs=1) as apool:
            # state arrives in 8 chunks of 4 k-tiles, interleaved with the
            # k-major fc1 weight chunks so fc1 m-group 0 starts immediately
            xall = apool.tile([P, 32 * ROWS], BF, name=f"{R}_xall")
            xall_v = xall.rearrange("p (k r) -> p k r", k=32)
            xTc = xT.rearrange("(j k) p r -> j p k r", j=8)
            wkt_sb = []
            for j in range(8):
                nc.sync.dma_start(xall_v[:, 4 * j:4 * j + 4, :], xTc[j])
                w_sb = wpool.tile([P, 4096], BF, tag="w1kt",
                                  name=f"{R}_w1kt_{j}")
                nc.sync.dma_start(w_sb[:], fc1kT_d[j])
                wkt_sb.append(w_sb)
            dummy_act(apool, AF.Tanh, "dtanh")   # pin a tanh+relu table
            cur = [xall_v[:, k, :] for k in range(32)]

            bias1_sb = apool.tile([P, 32], F32, name=f"{R}_bias0")
            nc.sync.dma_start(bias1_sb[:], fc_b[0][:])
            nxt0 = []
            with tc.tile_pool(name=f"{R}_ppF", bufs=1, space="PSUM") as ppF:
                pss = [ppF.tile([P, ROWS], F32, name=f"{R}_g0_{m}")
                       for m in range(8)]
                for k in range(32):
                    wv = wkt_sb[k // 4].rearrange("p (k m c) -> p k m c",
                                                  k=4, m=8)
                    for m in range(8):
                        nc.tensor.matmul(pss[m][:], wv[:, k % 4, m, :],
                                         cur[k][:], start=(k == 0),
                                         stop=(k == 31))
                for m in range(8):
                    o = apool.tile([P, ROWS], BF, name=f"{R}_a0_{m}")
                    nc.scalar.activation(o[:], pss[m][:], AF.Relu,
                                         bias=bias1_sb[:, m:m + 1])
                    nxt0.append(o)

            mpp_cm = tc.tile_pool(name=f"{R}_mlp_ps", bufs=3, space="PSUM")
            mpp = mpp_cm.__enter__()
            for i, (kin, mout) in enumerate(FC_DIMS):
                mt, kt = mout // P, kin // P
                if i == 0:
                    bias_sb = bias1_sb
                else:
                    bias_sb = apool.tile([P, mt], F32, name=f"{R}_bias{i}")
                    nc.sync.dma_start(bias_sb[:], fc_b[i][:])
                act = AF.Tanh if i == 4 else AF.Relu
                nxt = list(nxt0) if i == 0 else []
                for m in range(8 if i == 0 else 0, mt):
                    w_sb = wpool.tile([P, kt * P], BF, tag="wmlp",
                                      name=f"{R}_w{i}_{m}")
                    nc.sync.dma_start(w_sb[:], fc_w[i][m])
                    ps = mpp.tile([P, ROWS], F32, tag="mlp", name=f"{R}_ps{i}_{m}")
                    for k in range(kt):
                        nc.tensor.matmul(ps[:], w_sb[:, k * P:(k + 1) * P],
                                         cur[k][:], start=(k == 0),
                                         stop=(k == kt - 1))
                    o = apool.tile([P, ROWS], BF, name=f"{R}_a{i}_{m}")
                    nc.scalar.activation(o[:], ps[:], act,
                                         bias=bias_sb[:, m:m + 1])
                    nxt.append(o)
                cur = nxt

            # pre layer -> f32 stream + positional
            posT_sb = apool.tile([P, 2 * ROWS], F32, name=f"{R}_posT")
            posT_v = posT_sb.rearrange("p (m r) -> p m r", m=2)
            nc.sync.dma_start(posT_v[:], posT_d.rearrange("m p r -> p m r"))
            bias_sb = apool.tile([P, 2], F32, name=f"{R}_bias5")
            nc.sync.dma_start(bias_sb[:], fc_b[5][:])
            for m in range(2):
                w_sb = wpool.tile([P, 2 * P], BF, tag="wmlp", name=f"{R}_w5_{m}")
                nc.sync.dma_start(w_sb[:], fc_w[5][m])
                ps = mpp.tile([P, ROWS], F32, tag="mlp", name=f"{R}_ps5_{m}")
                for k in range(2):
                    nc.tensor.matmul(ps[:], w_sb[:, k * P:(k + 1) * P],
                                     cur[k][:], start=(k == 0), stop=(k == 1))
                nc.vector.scalar_tensor_tensor(
                    xs[m][:], ps[:], bias_sb[:, m:m + 1], posT_v[:, m, :],
                    op0=OP.add, op1=OP.add)
                if USE_POOL:
                    nc.gpsimd.tensor_copy(xbf[m][:], xs[m][:])
                else:
                    nc.vector.tensor_copy(xbf[m][:], xs[m][:])
            dummy_act(apool, AF.Sqrt, "dsqrt_front")  # prefetch sqrt table
            mpp_cm.__exit__(None, None, None)

        # ---------------- transformer blocks ----------------
        def rowmath(bpool, r2q, r2n, mu_ps, sq_ps, tag):
            """[1,512] row math. Writes [-mu; std] into r2q, nmr into
            r2n[0:1,:]; returns rstd_bf. rstd = 1/sqrt(var+eps) via Act
            Sqrt + DVE reciprocal (sqrt table prefetched by a dummy)."""
            musq = bpool.tile([1, ROWS], F32, tag=f"musq{tag}",
                              name=f"{R}_musq_{tag}")
            nc.scalar.activation(musq[:], mu_ps[:], AF.Square)
            var = bpool.tile([1, ROWS], F32, tag=f"var{tag}",
                             name=f"{R}_var_{tag}")
            nc.vector.tensor_tensor(var[:], sq_ps[:], musq[:], op=OP.subtract)
            nc.scalar.activation(r2q[32:33, :], var[:], AF.Sqrt,
                                 bias=LN_EPS)
            nc.scalar.activation(r2q[0:1, :], mu_ps[:], AF.Identity,
                                 scale=-1.0)
            rstd_f = bpool.tile([1, ROWS], F32, tag=f"rstdf{tag}",
                                name=f"{R}_rstdf_{tag}")
            nc.vector.reciprocal(rstd_f[:], r2q[32:33, :])
            if r2n is not None:
                nc.vector.scalar_tensor_tensor(r2n[0:1, :], mu_ps[:], -1.0,
                                               rstd_f[:], op0=OP.mult,
                                               op1=OP.mult)
            return rstd_f

        for l in range(NB):
            Rl = f"{R}_{l}"
            with tc.tile_pool(name=f"{Rl}_blk", bufs=1) as bpool:
                # ---- block weights (double-buffered in wpool across blocks)
                eqk_sb = wpool.tile([P, 1024], BF, tag="eqk", bufs=2,
                                    name=f"{Rl}_eqk")
                nc.sync.dma_start(eqk_sb[:], encqk_d[l])
                qkr2_sb = wpool.tile([33, 512], BF, tag="qkr2", bufs=2,
                                     name=f"{Rl}_qkr2")
                nc.sync.dma_start(qkr2_sb[:], qkr2_d[l])
                ev_sb = wpool.tile([P, 512], BF, tag="ev", bufs=2,
                                   name=f"{Rl}_ev")
                nc.sync.dma_start(ev_sb[:], encv_d[l])
                vr2_sb = wpool.tile([33, 256], BF, tag="vr2", bufs=2,
                                    name=f"{Rl}_vr2")
                nc.sync.dma_start(vr2_sb[:], vr2_d[l])
                rw1_sb = wpool.tile([P, 2048], BF, tag="rw1", bufs=2,
                                    name=f"{Rl}_rw1")
                nc.sync.dma_start(rw1_sb[:], rw1_d[l])
                f1r2_sb = wpool.tile([33, 1024], BF, tag="f1r2", bufs=2,
                                     name=f"{Rl}_f1r2")
                nc.sync.dma_start(f1r2_sb[:], f1r2_d[l])
                rw2_sb = wpool.tile([P, 2048], BF, tag="rw2", bufs=2,
                                    name=f"{Rl}_rw2")
                nc.sync.dma_start(rw2_sb[:], rw2_d[l])
                rb2_sb = wpool.tile([P, 2], F32, tag="rb2", bufs=2,
                                    name=f"{Rl}_rb2")
                nc.sync.dma_start(rb2_sb[:], rb2_d[l])
                g2b2_sb = wpool.tile([33, 256], BF, tag="g2b2", bufs=2,
                                     name=f"{Rl}_g2b2")
                nc.sync.dma_start(g2b2_sb[:], g2b2_d[l])
                g2f_sb = wpool.tile([1, 256], F32, tag="g2f", bufs=2,
                                    name=f"{Rl}_g2f")
                nc.sync.dma_start(g2f_sb[:], g2f_d[l])

                eqk_v = eqk_sb.rearrange("p (m k c) -> p m k c", m=4, k=2)
                ev_v = ev_sb.rearrange("p (k c) -> p k c", k=2)
                rw1_v = rw1_sb.rearrange("p (m k c) -> p m k c", m=8, k=2)
                rw2_v = rw2_sb.rearrange("p (m k c) -> p m k c", m=2, k=8)

                # ---- LN1 stats + QKV + V
                qk_bf, va = [], []
                with tc.tile_pool(name=f"{Rl}_ppA", bufs=1,
                                  space="PSUM") as ppA:
                    x2 = []
                    for m in range(2):
                        t = bpool.tile([P, ROWS], BF, tag=f"x2_{m}",
                                       name=f"{Rl}_x2_{m}")
                        if USE_POOL:
                            nc.gpsimd.tensor_tensor(t[:], xbf[m][:],
                                                    xbf[m][:], op=OP.mult)
                        else:
                            nc.vector.tensor_tensor(t[:], xbf[m][:],
                                                    xbf[m][:], op=OP.mult)
                        x2.append(t)
                    st_ps = ppA.tile([P, ROWS], F32, name=f"{Rl}_st1")
                    mu_ps, sq_ps = st_ps[0:1, :], st_ps[32:33, :]
                    for m in range(2):
                        nc.tensor.matmul(mu_ps, m256_sb[:], xbf[m][:],
                                         start=(m == 0), stop=(m == 1))
                        nc.tensor.matmul(sq_ps, m256_sb[:], x2[m][:],
                                         start=(m == 0), stop=(m == 1))
                    # QKV main matmuls on xbf (no rowmath dependency):
                    # W^T(x*rstd) == (W^T x)*rstd
                    c_ps = []
                    for mt in range(4):
                        ps = ppA.tile([P, ROWS], F32, tag="c", bufs=4,
                                      name=f"{Rl}_c_{mt}")
                        for k in range(2):
                            nc.tensor.matmul(ps[:], eqk_v[:, mt, k, :],
                                             xbf[k][:], start=(k == 0),
                                             stop=False)
                        c_ps.append(ps)
                    rstd_f = rowmath(bpool, r2qa, None, mu_ps, sq_ps,
                                     f"1_{l}")
                    dummy_act(bpool, AF.Exp, f"dexp_{l}")
                    bca = ppA.tile([P, ROWS], F32, name=f"{Rl}_bca")
                    nc.tensor.matmul(bca[:], nc.onesF[0:1, :], rstd_f[:],
                                     start=True, stop=True)
                    bca_sb = bpool.tile([P, ROWS], BF, tag="bca_sb",
                                        name=f"{Rl}_bca_sb")
                    nc.scalar.copy(bca_sb[:], bca[:])
                    # rstd transposed to columns (reuses st_ps bank):
                    # cols 0-3 rstd (V row-scale), 4-7 SCALE*rstd (exp scale)
                    rcol_ps = st_ps[:, 504:508]
                    for t in range(4):
                        nc.tensor.matmul(rcol_ps[:, t:t + 1],
                                         rstd_f[0:1, t * P:(t + 1) * P],
                                         nc.onesF[0:1, 0:1],
                                         is_transpose=True,
                                         skip_group_check=True)
                    rcol_sb = bpool.tile([P, 8], F32, tag="rcol",
                                         name=f"{Rl}_rcol")
                    nc.vector.tensor_copy(rcol_sb[:, 0:4], rcol_ps)
                    nc.vector.tensor_scalar(rcol_sb[:, 4:8], rcol_ps, SCALE,
                                            None, op0=OP.mult)
                    # rank-2 [-mu; std]; post-scale Q by bc(rstd) on DVE,
                    # K copied unscaled (rstd[key] folds into the exp scale)
                    for mt in range(4):
                        nc.tensor.matmul(c_ps[mt][:],
                                         qkr2_sb[:, mt * P:(mt + 1) * P],
                                         r2qa[:], start=False, stop=True)
                        o = bpool.tile([P, ROWS], BF, tag=f"qk_{mt}",
                                       name=f"{Rl}_qkbf_{mt}")
                        if mt < 2:
                            nc.vector.tensor_tensor(o[:], c_ps[mt][:],
                                                    bca_sb[:], op=OP.mult)
                        else:
                            nc.scalar.copy(o[:], c_ps[mt][:])
                        qk_bf.append(o)
                    # V natural [keys, dims] from xbf + [-mu; std] rank-2,
                    # then row-scale by rstd[key] + aug ones column
                    for rt in range(4):
                        ps = ppA.tile([P, 256], F32, tag="v", bufs=2,
                                      name=f"{Rl}_v_{rt}")
                        for k in range(2):
                            nc.tensor.matmul(
                                ps[:], xbf[k][:, rt * P:(rt + 1) * P],
                                ev_v[:, k, :], start=(k == 0), stop=False)
                        nc.tensor.matmul(ps[:], r2qa[:, rt * P:(rt + 1) * P],
                                         vr2_sb[:], start=False, stop=True)
                        t = bpool.tile([P, 264], BF, tag=f"va_{rt}",
                                       name=f"{Rl}_va_{rt}")
                        t_v = t.rearrange("p (h c) -> p h c", c=33)
                        nc.vector.tensor_scalar(
                            t_v[:, :, 0:32],
                            ps.rearrange("p (h c) -> p h c", c=32),
                            rcol_sb[:, rt:rt + 1], None, op0=OP.mult)
                        nc.vector.memset(t_v[:, :, 32:33], 1.0)
                        va.append(t)

                # ---- attention, two head-groups interleaved
                with tc.tile_pool(name=f"{Rl}_ppB", bufs=1,
                                  space="PSUM") as ppB:
                    expS = {}
                    for g in range(2):
                        for hh in range(4):
                            for t in range(4):
                                w = t * P
                                s_ps = ppB.tile([P, ROWS], F32, tag="s",
                                                bufs=4,
                                                name=f"{Rl}_s_{g}_{hh}_{t}")
                                nc.tensor.matmul(
                                    s_ps[:, w:],
                                    qk_bf[2 + g][32 * hh:32 * hh + 32,
                                                 w:w + P],
                                    qk_bf[g][32 * hh:32 * hh + 32, w:],
                                    start=True, stop=True,
                                    tile_position=(32 * hh, 0))
                                e = bpool.tile([P, ROWS], BF,
                                               tag=f"e_{hh}_{t}", bufs=2,
                                               name=f"{Rl}_e_{g}_{hh}_{t}")
                                nc.scalar.activation(
                                    e[:, w:], s_ps[:, w:], AF.Exp,
                                    scale=rcol_sb[:, 4 + t:5 + t])
                                if USE_POOL:
                                    nc.gpsimd.tensor_tensor(
                                        e[:, w:w + P], e[:, w:w + P],
                                        mask_sb[:], op=OP.mult)
                                else:
                                    nc.vector.tensor_tensor(
                                        e[:, w:w + P], e[:, w:w + P],
                                        mask_sb[:], op=OP.mult)
                                expS[(g, hh, t)] = e
                    dummy_act(bpool, AF.Sqrt, f"dsqrt2_{l}")

                    pv_tiles, r_tiles = {}, {}

                    def emit_pv(g):
                        for pi in range(2):
                            pv = ppB.tile([P, ROWS], F32, tag="pv", bufs=2,
                                          name=f"{Rl}_pv_{g}_{pi}")
                            gA, gB = 4 * g + 2 * pi, 4 * g + 2 * pi + 1
                            for t in range(4):
                                w = t * P
                                nc.tensor.matmul(
                                    pv[0:33, w:],
                                    va[t][:, 33 * gA:33 * gA + 33],
                                    expS[(g, 2 * pi, t)][:, w:],
                                    start=(t == 0), stop=(t == 3),
                                    tile_position=(0, 0),
                                    skip_group_check=True)
                                nc.tensor.matmul(
                                    pv[64:97, w:],
                                    va[t][:, 33 * gB:33 * gB + 33],
                                    expS[(g, 2 * pi + 1, t)][:, w:],
                                    start=(t == 0), stop=(t == 3),
                                    tile_position=(0, 64),
                                    skip_group_check=True)
                            pv_tiles[(g, pi)] = pv

                    def emit_rbc(g):
                        for pi in range(2):
                            pv = pv_tiles[(g, pi)]
                            dpair = dpairs[pi]
                            with nc.allow_low_precision(
                                    reason="softmax denom recip in bf16"):
                                nc.vector.reciprocal(dpair[0:1, :],
                                                     pv[32:33, :])
                                nc.vector.reciprocal(dpair[32:33, :],
                                                     pv[96:97, :])
                            r_ps = ppB.tile([P, ROWS], F32, tag="rb", bufs=1,
                                            name=f"{Rl}_rb_{g}_{pi}")
                            nc.tensor.matmul(r_ps[:], rpat_sb[:], dpair[:],
                                             start=True, stop=True)
                            rbf = bpool.tile([P, ROWS], BF, tag=f"rbf_{pi}",
                                             name=f"{Rl}_rbf_{g}_{pi}")
                            nc.scalar.copy(rbf[:], r_ps[:])
                            r_tiles[(g, pi)] = rbf

                    def emit_at(g):
                        at = bpool.tile([P, ROWS], F32, tag=f"at_{g}",
                                        name=f"{Rl}_at_{g}")
                        for pi in range(2):
                            pv, rbf = pv_tiles[(g, pi)], r_tiles[(g, pi)]
                            for j, band in ((0, 0), (1, 64)):
                                q = 2 * pi + j
                                nc.vector.tensor_tensor(
                                    at[32 * q:32 * q + 32, :],
                                    pv[band:band + 32, :],
                                    rbf[band:band + 32, :], op=OP.mult)
                        nc.vector.tensor_tensor(xs[g][:], xs[g][:], at[:],
                                                op=OP.add)

                    # LN2 stats interleaved with the attention tail: tile m
                    # is final right after emit_at(m)
                    st2_ps = ppB.tile([P, ROWS], F32, name=f"{Rl}_st2")
                    mu2, sq2 = st2_ps[0:1, :], st2_ps[32:33, :]
                    xbf2, x22 = [None, None], [None, None]

                    def emit_ln2_stats(m):
                        t = bpool.tile([P, ROWS], BF, tag=f"xbf2_{m}",
                                       name=f"{Rl}_xbf2_{m}")
                        t2 = bpool.tile([P, ROWS], BF, tag=f"x22_{m}",
                                        name=f"{Rl}_x22_{m}")
                        if USE_POOL:
                            nc.gpsimd.tensor_copy(t[:], xs[m][:])
                            nc.gpsimd.tensor_tensor(t2[:], t[:], t[:],
                                                    op=OP.mult)
                        else:
                            nc.vector.tensor_copy(t[:], xs[m][:])
                            nc.vector.tensor_tensor(t2[:], t[:], t[:],
                                                    op=OP.mult)
                        xbf2[m], x22[m] = t, t2
                        nc.tensor.matmul(mu2, m256_sb[:], t[:],
                                         start=(m == 0), stop=(m == 1))
                        nc.tensor.matmul(sq2, m256_sb[:], t2[:],
                                         start=(m == 0), stop=(m == 1))

                    emit_pv(0)
                    emit_rbc(0)
                    emit_pv(1)
                    emit_at(0)
                    emit_rbc(1)
                    emit_at(1)
                    emit_ln2_stats(0)
                    emit_ln2_stats(1)
                    rstd2_f = rowmath(bpool, r2qb, r2b, mu2, sq2, f"2_{l}")
                    dummy_act(bpool, AF.Gelu, f"dgelu_{l}")

                # ---- LN2 + FFN + stream rebuild
                with tc.tile_pool(name=f"{Rl}_ppD", bufs=1,
                                  space="PSUM") as ppD:
                    # FFN1 main matmuls (overlap the rowmath above)
                    y_ps = []
                    for mt in range(8):
                        ps = ppD.tile([P, ROWS], F32, tag="ff", bufs=4,
                                      name=f"{Rl}_y_{mt}")
                        for k in range(2):
                            nc.tensor.matmul(ps[:], rw1_v[:, mt, k, :],
                                             xbf2[k][:], start=(k == 0),
                                             stop=False)
                        y_ps.append(ps)
                    bca2 = ppD.tile([P, ROWS], F32, name=f"{Rl}_bca2")
                    nc.tensor.matmul(bca2[:], nc.onesF[0:1, :], rstd2_f[:],
                                     start=True, stop=True)
                    bca2_sb = bpool.tile([P, ROWS], BF, tag="bca2_sb",
                                         name=f"{Rl}_bca2_sb")
                    nc.scalar.copy(bca2_sb[:], bca2[:])
                    h = []
                    for mt in range(8):
                        nc.tensor.matmul(y_ps[mt][:],
                                         f1r2_sb[:, mt * P:(mt + 1) * P],
                                         r2qb[:], start=False, stop=True)
                        gi = bpool.tile([P, ROWS], BF, tag=f"gi_{mt % 4}",
                                        name=f"{Rl}_gi_{mt}")
                        nc.vector.tensor_tensor(gi[:], y_ps[mt][:],
                                                bca2_sb[:], op=OP.mult)
                        o = bpool.tile([P, ROWS], BF, tag=f"h_{mt}",
                                       name=f"{Rl}_h_{mt}")
                        nc.scalar.activation(o[:], gi[:], AF.Gelu)
                        h.append(o)
                    dummy_act(bpool, AF.Sqrt, f"dsqrt_{l}")
                    # stream rebuild: xs = xs*(rstd2*g2) + [g2*nmr2 + b2 +
                    # ffn2_out + rb2]; the bracket accumulates in the FFN2
                    # PSUM via a rank-2 matmul.
                    for m in range(2):
                        a_ps = ppD.tile([P, ROWS], F32, tag="ab",
                                        bufs=2, name=f"{Rl}_a_{m}")
                        nc.tensor.matmul(a_ps[:],
                                         g2f_sb[0:1, m * P:(m + 1) * P],
                                         rstd2_f[:], start=True,
                                         stop=True)
                        ps = ppD.tile([P, ROWS], F32, tag="ff", bufs=4,
                                      name=f"{Rl}_f2_{m}")
                        for k in range(8):
                            nc.tensor.matmul(ps[:], rw2_v[:, m, k, :],
                                             h[k][:], start=(k == 0),
                                             stop=False)
                        nc.tensor.matmul(ps[:],
                                         g2b2_sb[:, m * P:(m + 1) * P],
                                         r2b[:], start=False, stop=True)
                        t1 = bpool.tile([P, ROWS], F32, tag=f"t1_{m}",
                                        name=f"{Rl}_t1_{m}")
                        nc.vector.tensor_tensor(t1[:], xs[m][:], a_ps[:],
                                                op=OP.mult)
                        nc.vector.scalar_tensor_tensor(
                            xs[m][:], ps[:], rb2_sb[:, m:m + 1], t1[:],
                            op0=OP.add, op1=OP.add)
                        if USE_POOL:
                            nc.gpsimd.tensor_copy(xbf[m][:], xs[m][:])
                        else:
                            nc.vector.tensor_copy(xbf[m][:], xs[m][:])

        # ---------------- output head + global standardize ----------------
        with tc.tile_pool(name=f"{R}_fin", bufs=1) as fpool, \
             tc.tile_pool(name=f"{R}_fin_ps", bufs=1, space="PSUM") as opp:
            outw_sb = fpool.tile([P, 256], BF, name=f"{R}_outw")
            nc.sync.dma_start(outw_sb[:], outw_d[:])
            outb_sb = fpool.tile([P, 1], F32, name=f"{R}_outb")
            nc.sync.dma_start(outb_sb[:], outb_d[:])
            ops = opp.tile([P, ROWS], F32, name=f"{R}_out_ps")
            for k in range(2):
                nc.tensor.matmul(ops[:], outw_sb[:, k * P:(k + 1) * P],
                                 xbf[k][:], start=(k == 0), stop=(k == 1))
            out_sb = fpool.tile([P, ROWS], F32, name=f"{R}_out_sb")
            nc.scalar.activation(out_sb[:], ops[:], AF.Identity,
                                 bias=outb_sb[:, 0:1])
            sc = fpool.tile([P, 2], F32, name=f"{R}_sc")
            nc.vector.tensor_reduce(sc[:, 0:1], out_sb[:],
                                    axis=mybir.AxisListType.X, op=OP.add)
            sq_scr = fpool.tile([P, ROWS], F32, name=f"{R}_sq_scr")
            nc.scalar.activation(sq_scr[:], out_sb[:], AF.Square,
                                 accum_out=sc[:, 1:2])
            scbf = fpool.tile([P, 2], BF, name=f"{R}_scbf")
            nc.vector.tensor_copy(scbf[:], sc[:])
            tot_ps = opp.tile([1, 2], F32, name=f"{R}_tot_ps")
            nc.tensor.matmul(tot_ps[:], onesP_sb[:], scbf[:],
                             start=True, stop=True)

            tot_sb = fpool.tile([1, 2], F32, name=f"{R}_tot_sb")
            nc.vector.tensor_copy(tot_sb[:], tot_ps[:])
            if n_cores > 1:
                cc_in = dpool.tile([1, 2], F32, name=f"{R}_cc_in")
                cc_out = dpool.tile([1, 2], F32, addr_space="Shared",
                                    name=f"{R}_cc_out")
                nc.sync.dma_start(cc_in[:], tot_sb[:])
                nc.gpsimd.collective_compute(
                    "AllReduce", OP.add,
                    replica_groups=[list(range(n_cores))],
                    ins=[cc_in[:]], outs=[cc_out[:]])
                st_sb = fpool.tile([1, 2], F32, name=f"{R}_st_sb")
                nc.sync.dma_start(st_sb[:], cc_out[:])
            else:
                st_sb = tot_sb

            mean = fpool.tile([1, 1], F32, name=f"{R}_mean")
            nc.vector.tensor_scalar(mean[:], st_sb[:, 0:1], 1.0 / NTOT,
                                    None, op0=OP.mult)
            tb = fpool.tile([1, 1], F32, name=f"{R}_tb")
            nc.vector.tensor_tensor(tb[:], mean[:], mean[:], op=OP.mult)
            ta = fpool.tile([1, 1], F32, name=f"{R}_ta")
            nc.vector.tensor_scalar(ta[:], st_sb[:, 1:2],
                                    1.0 / (NTOT - 1.0), None, op0=OP.mult)
            var = fpool.tile([1, 1], F32, name=f"{R}_var")
            nc.vector.scalar_tensor_tensor(
                var[:], tb[:], -NTOT / (NTOT - 1.0), ta[:],
                op0=OP.mult, op1=OP.add)
            stdv = fpool.tile([1, 1], F32, name=f"{R}_stdv")
            nc.scalar.activation(stdv[:], var[:], AF.Sqrt)
            rs_pack = fpool.tile([1, 2], F32, name=f"{R}_rs_pack")
            nc.vector.reciprocal(rs_pack[:, 0:1], stdv[:])
            tshift = fpool.tile([1, 1], F32, name=f"{R}_tshift")
            nc.vector.scalar_tensor_tensor(
                tshift[:], mean[:], -1.0, rs_pack[:, 0:1],
                op0=OP.mult, op1=OP.mult)
            nc.vector.tensor_scalar(rs_pack[:, 1:2], tshift[:], 1e-10,
                                    None, op0=OP.add)
            bc = fpool.tile([P, 2], F32, name=f"{R}_bc")
            nc.gpsimd.partition_broadcast(bc[:], rs_pack[:])
            nc.vector.tensor_scalar(out_sb[:], out_sb[:], bc[:, 0:1],
                                    bc[:, 1:2], op0=OP.mult, op1=OP.add)
            nc.sync.dma_start(out_d[:], out_sb[:])


# ---------------- host-side weight prep ----------------

def _bf(a):
    return np.ascontiguousarray(a).astype(ml_dtypes.bfloat16)


def _f32(a):
    return np.ascontiguousarray(a, dtype=np.float32)


def _tile_w(w):
    """[K, M] -> [Mt, 128, Kt*128] with sb[m, p, k*128+c] = w[k*128+p, m*128+c]."""
    K, M = w.shape
    kt, mt = K // P, M // P
    return _bf(w.reshape(kt, P, mt, P).transpose(2, 1, 0, 3).reshape(mt, P, kt * P))


def _bias_grid(b):
    M = b.shape[0]
    return _f32(np.asarray(b).reshape(M // P, P).T)


def prep_shared(inp):
    d = {}
    for i, name in enumerate(["fc1", "fc2", "fc3", "fc4", "fc5"]):
        d[f"w{i}"] = _tile_w(np.asarray(inp[f"{name}_w"]))
        d[f"b{i}"] = _bias_grid(np.asarray(inp[f"{name}_b"]))
    d["w5"] = _tile_w(np.asarray(inp["pre_w"]))
    d["b5"] = _bias_grid(np.asarray(inp["pre_b"]))
    d["posT"] = _f32(np.asarray(inp["pos_w"])[0].T.reshape(2, P, ROWS))

    enc_w = np.asarray(inp["enc_w"], dtype=np.float64)   # [NB, 256, 768]
    enc_b = np.asarray(inp["enc_b"], dtype=np.float64)   # [NB, 768]
    g1 = np.asarray(inp["ln1_g"], dtype=np.float64)      # [NB, 256]
    b1 = np.asarray(inp["ln1_b"], dtype=np.float64)
    g2 = np.asarray(inp["ln2_g"], dtype=np.float64)
    b2 = np.asarray(inp["ln2_b"], dtype=np.float64)
    res_w1 = np.asarray(inp["res_w1"], dtype=np.float64)  # [NB, 256, 1024]
    res_b1 = np.asarray(inp["res_b1"], dtype=np.float64)
    res_w2 = np.asarray(inp["res_w2"])                    # [NB, 1024, 256]
    res_b2 = np.asarray(inp["res_b2"])

    Wg = enc_w * g1[:, :, None]                           # [NB, 256, 768]
    bfold = np.einsum("nd,ndm->nm", b1, enc_w) + enc_b    # [NB, 768]
    d["encqk"] = _bf(Wg[:, :, :512].reshape(NB, 2, P, 4, P)
                     .transpose(0, 2, 3, 1, 4).reshape(NB, P, 1024))
    d["encv"] = _bf(Wg[:, :, 512:].reshape(NB, 2, P, 256)
                    .transpose(0, 2, 1, 3).reshape(NB, P, 512))
    def _r33(u, b):
        z = np.zeros((NB, 33, u.shape[1]))
        z[:, 0], z[:, 32] = u, b
        return _bf(z)

    d["qkr2"] = _r33(Wg[:, :, :512].sum(axis=1), bfold[:, :512])
    d["vr2"] = _r33(Wg[:, :, 512:].sum(axis=1), bfold[:, 512:])

    W1g = res_w1 * g2[:, :, None]                         # [NB, 256, 1024]
    b1fold = np.einsum("nd,ndm->nm", b2, res_w1) + res_b1  # [NB, 1024]
    d["rw1"] = _bf(W1g.reshape(NB, 2, P, 8, P).transpose(0, 2, 3, 1, 4)
                   .reshape(NB, P, 2048))
    d["f1r2"] = _r33(W1g.sum(axis=1), b1fold)
    d["rw2"] = _bf(res_w2.reshape(NB, 8, P, 2, P).transpose(0, 2, 3, 1, 4)
                   .reshape(NB, P, 2048))
    d["rb2"] = _f32(res_b2.reshape(NB, 2, P).transpose(0, 2, 1))
    d["g2b2r"] = _r33(g2, b2)
    d["g2f"] = _f32(g2.reshape(NB, 1, 256))
    w1 = np.asarray(inp["fc1_w"])                        # [4096, 4096]
    d["fc1kT"] = _bf(w1.reshape(32, P, 32, P)[:, :, :8, :]
                     .reshape(8, 4, P, 8 * P).transpose(0, 2, 1, 3)
                     .reshape(8, P, 4096))

    ow = np.asarray(inp["out_w"])  # [256, 128]
    d["outw"] = _bf(ow.reshape(2, P, P).transpose(1, 0, 2).reshape(P, 256))
    d["outb"] = _f32(np.asarray(inp["out_b"]).reshape(P, 1))

    jj = np.arange(P)[:, None]   # key (partition)
    ii = np.arange(P)[None, :]   # query (free)
    d["maskbf"] = _bf(np.where(ii >= jj, 1.0, 0.0))
    d["m256"] = _bf(np.full((P, 1), 1.0 / 256.0))
    d["onesP"] = _bf(np.ones((P, 1)))
    d["lnones"] = _bf(np.ones((1, ROWS)))
    rpat = np.zeros((33, P), np.float32)
    rpat[0, 0:32] = 1.0
    rpat[32, 64:96] = 1.0
    d["rpat"] = _bf(rpat)
    return d


_CACHED_NC = None
TRACE = False
LAST_RESULT = None
LAST_IN_MAPS = None


def kernel(**inputs) -> np.ndarray:
    global _CACHED_NC, LAST_RESULT, LAST_IN_MAPS
    if _CACHED_NC is None:
        _CACHED_NC = build_nc(8)
    nc = _CACHED_NC

    shared = prep_shared(inputs)
    state = np.asarray(inputs["state"], dtype=np.float32).reshape(B, S, 4096)
    in_maps = []
    for b in range(B):
        m = dict(shared)
        m["xT"] = _bf(state[b].T.reshape(32, P, ROWS))
        in_maps.append(m)
    LAST_IN_MAPS = in_maps

    res = run_bass_kernel_spmd(nc, in_maps, core_ids=list(range(8)),
                               trace=TRACE)
    LAST_RESULT = res
    out = np.stack([res.results[i]["out"] for i in range(B)])  # [B, COMP, S]
    return np.ascontiguousarray(out.transpose(0, 2, 1)).astype(np.float32)
